# revision 1
# baseline (speedup 1.0000x reference)
"""Trainium2 Bass kernel for nn_DifferentialMaxtree (v2).

Strategy (8 NeuronCores, data-parallel over the 32 (b,n) trees, 4 per core):
  1. Features/logits/sigmoid/w = diff*score/SCALING on ACT+DVE (fp32).
  2. Path sums via pointer doubling with sigma=8 subtable gathers:
     - gather tables live in SBUF [128, 8192] f32 (16 replicas of the
       65536-entry image, partition t holds entries [(t%8)*8192, ...));
     - tables built by 4 parallel SBUF->DRAM writes of the s image plus
       4 parallel 1MB DRAM->SBUF replica reads (no DRAM roundtrip chains);
     - candidate co-location ON-CHIP via the DVE 32x32 StreamTranspose
       (no DRAM bounce), then an 8-way masked select with fused
       scalar_tensor_tensor ops; sentinel/pad slots use qsel=99 -> add 0.
     - components are depth-sorted so iteration k only gathers the
       active prefix (depth >= 2^k).
  3. Pixel lookups gather from the final val table; requests are sorted
     host-side by subtable-row q so results are written straight from the
     gather output to DRAM with 8 partition-strided DMAs per instruction
     (no transpose, no select). Host inverts all permutations.

All floating point math happens on device; the host only does integer
index bookkeeping (chains, depths, sorts, wrap16 layouts).
"""

import numpy as np

import concourse.bass as bass
import concourse.bacc as bacc
import concourse.mybir as mybir
import concourse.tile as tile
from concourse.bass_utils import run_bass_kernel_spmd

f32 = mybir.dt.float32
fp16 = mybir.dt.float16
i16 = mybir.dt.int16
Alu = mybir.AluOpType
Act = mybir.ActivationFunctionType

CFG = dict(
    B=4, N=8, H=512, W=512, C=65536,
    NCORES=8, TPC=4, P=128, J=512,
    NE=8192,          # table entries per partition (sigma=8)
    RG=1056,          # pixel request slots per (core, q-group) per instr
    NPIX_INSTR=4,     # pixel gather instructions per tree
    EPS=1e-10, SCALING=10.0,
)
PCOLS = 8 * CFG["RG"]          # pixel gather columns per core per instr


# ---------------------------------------------------------------- host prep


def _host_prep(cfg, diff, attrs, weight, bias, parent, pix2cc):
    B, N, C, P, J = cfg["B"], cfg["N"], cfg["C"], cfg["P"], cfg["J"]
    NCORES, TPC, NE, RG = cfg["NCORES"], cfg["TPC"], cfg["NE"], cfg["RG"]
    NPI = cfg["NPIX_INSTR"]
    NPX = cfg["H"] * cfg["W"]

    # ---- pointer chains (sentinel C) --------------------------------------
    pz = np.concatenate([parent, np.full((B, N, 1), C, np.int32)], axis=-1)
    chains = []
    cur = pz.copy()
    k_iters = 0
    for _ in range(17):
        if (cur[..., :C] == C).all():
            break
        chains.append(cur[..., :C].copy())
        cur = np.take_along_axis(cur, cur, axis=-1)
        k_iters += 1
    if k_iters == 0:
        chains.append(cur[..., :C].copy())
        k_iters = 1

    # ---- depth via binary chain walk --------------------------------------
    depth = np.zeros((B, N, C), np.int64)
    cur_node = np.broadcast_to(np.arange(C), (B, N, C)).copy()
    for k in range(k_iters - 1, -1, -1):
        ch = np.concatenate([chains[k], np.full((B, N, 1), C, np.int32)],
                            axis=-1)
        nxt = np.take_along_axis(ch, cur_node, axis=-1)
        mask = nxt != C
        depth += mask << k
        cur_node = np.where(mask, nxt, cur_node)

    # ---- slot assignment: rank by depth desc, deal round-robin ------------
    # rank rho: 0 = deepest. home partition p = rho % 128, slot j = rho // 128
    order = np.argsort(-depth, axis=-1, kind="stable")      # [B,N,C] comp ids
    rho = np.empty_like(order)
    ar = np.arange(C)
    for b in range(B):
        for n in range(N):
            rho[b, n, order[b, n]] = ar
    home_p = rho % P                                         # [B,N,C]
    home_j = rho // P
    tpos = home_p * J + home_j                               # table position

    # active slot counts per iteration (global max over trees/partitions)
    n_slots = []
    for k in range(k_iters):
        n_act = (depth >= (1 << k)).sum(axis=-1)             # [B,N]
        mx = int(np.ceil(n_act.max() / P))
        mx = max(2, mx + (mx & 1))                           # even, >=2
        mx = min(mx, J)
        n_slots.append(mx)

    # ---- request/tile layout helpers --------------------------------------
    # slot (p, j) -> request (core, i): core = 2*(p//32) + j%2,
    #                                   i = 32*(j//2) + p%32
    # request (core, i) -> idx tile cell (16*core + i%16, i//16)
    pgrid = np.arange(P)[:, None]
    tpos_pad = np.concatenate(
        [tpos, np.zeros((B, N, 1), tpos.dtype)], axis=-1)    # tpos[C] unused pad

    in_maps = [dict() for _ in range(NCORES)]

    # ---- per-tree chase inputs --------------------------------------------
    # packed per (tree, iter): i16 [P, 2*ns]: [:, :ns] idx, [:, ns:] qsel fp16
    for core in range(NCORES):
        packs = []
        for k in range(k_iters):
            ns = n_slots[k]
            pk = np.zeros((TPC, P, 2 * ns), np.int16)
            packs.append(pk)
        in_maps[core]["pk"] = packs

    # comp_at[p, j] = component id at slot (p, j), per tree
    for t in range(B * N):
        b, n = t // N, t % N
        core, j_tree = t // TPC, t % TPC
        comp_at = np.empty((P, J), np.int64)
        comp_at[home_p[b, n], home_j[b, n]] = np.arange(C)
        for k in range(k_iters):
            ns = n_slots[k]
            jgrid = np.arange(ns)[None, :]
            comps = comp_at[:, :ns]                          # [P, ns]
            targ = chains[k][b, n][comps]                    # [P, ns] in [0,C]
            act = targ != C
            tp = tpos_pad[b, n][np.where(act, targ, C)]
            qv = np.where(act, tp // NE, 99).astype(np.float16)
            iv = np.where(act, tp % NE, 0).astype(np.int16)
            # scatter into wrapped idx tile + qsel plane
            kcore = 2 * (pgrid // 32) + (jgrid % 2)          # [P, ns]
            i_req = 32 * (jgrid // 2) + (pgrid % 32)
            tile_p = 16 * kcore + (i_req % 16)
            tile_o = i_req // 16
            pk = in_maps[core]["pk"][k][j_tree]
            pk[tile_p, tile_o] = iv
            pk[:, ns:] = qv.view(np.int16)

    # ---- features inputs ---------------------------------------------------
    for core in range(NCORES):
        attrs_w = np.empty((TPC, P, J, 15), np.float32)
        diff_w = np.empty((TPC, P, J), np.float32)
        wgB = np.empty((P, TPC, 17), np.float32)
        biasB = np.empty((P, TPC), np.float32)
        for j_tree in range(TPC):
            t = core * TPC + j_tree
            b, n = t // N, t % N
            comp_at = np.empty((P, J), np.int64)
            comp_at[home_p[b, n], home_j[b, n]] = np.arange(C)
            attrs_w[j_tree] = attrs[b, n][comp_at]
            diff_w[j_tree] = diff[b, n][comp_at]
            wgB[:, j_tree, :] = weight[n, :, 0][None, :]
            biasB[:, j_tree] = bias[n, 0]
        in_maps[core]["attrs_w"] = attrs_w
        in_maps[core]["diff_w"] = diff_w
        in_maps[core]["wgB"] = wgB
        in_maps[core]["biasB"] = biasB

    # permutation weights for table-build matmuls:
    # table[t, r*J + e0] = s[(t%8)*16 + r, e0]  ->  W_r[pi, po] = [pi == (po%8)*16+r]
    wperm = np.zeros((P, 16, P), np.float16)
    po = np.arange(P)
    for r in range(16):
        wperm[(po % 8) * 16 + r, r, po] = 1.0
    for core in range(NCORES):
        in_maps[core]["wperm"] = wperm

    # ---- pixel inputs ------------------------------------------------------
    # per (tree, instr): idx i16 [P, PCOLS//16]; assembly map per tree
    asm = []   # per core: list of (lin_idx_into_pixout, orig_positions)
    for core in range(NCORES):
        p_idx = np.zeros((TPC * NPI, P, PCOLS // 16), np.int16)
        lin_all = []
        for j_tree in range(TPC):
            t = core * TPC + j_tree
            b, n = t // N, t % N
            pix = pix2cc[b, n].reshape(-1)                   # [NPX]
            tp = tpos[b, n][pix]                             # [NPX]
            q = tp // NE
            order_px = np.argsort(q, kind="stable")
            q_s = q[order_px]
            tp_s = tp[order_px]
            seq = np.arange(NPX) - np.searchsorted(q_s, q_s)  # rank in q-run
            kcore = seq % 8
            instr = (seq // 8) % NPI
            pos = seq // (8 * NPI)
            assert pos.max() < RG, f"pixel group overflow {pos.max()}"
            col = q_s * RG + pos
            tile_p = 16 * kcore + (col % 16)
            tile_o = col // 16
            p_idx[j_tree * NPI + instr, tile_p, tile_o] = (tp_s % NE).astype(
                np.int16)
            # pixout layout: [TPC*NPI, 8, 8, RG] = (instr, q, core, pos)
            lin = (((j_tree * NPI + instr) * 8 + q_s) * 8 + kcore) * RG + pos
            lin_all.append((lin, order_px))
        in_maps[core]["p_idx"] = p_idx
        asm.append(lin_all)

    meta = dict(k_iters=k_iters, n_slots=tuple(n_slots), asm=asm)
    return in_maps, meta


def _host_assemble(cfg, results, meta):
    B, N, H, W = cfg["B"], cfg["N"], cfg["H"], cfg["W"]
    NCORES, TPC = cfg["NCORES"], cfg["TPC"]
    out = np.empty((B, N, H, W), np.float32)
    for core in range(NCORES):
        po = results[core]["pixout"].reshape(-1)
        for j_tree in range(TPC):
            t = core * TPC + j_tree
            b, n = t // N, t % N
            lin, order_px = meta["asm"][core][j_tree]
            flat = np.empty(H * W, np.float32)
            flat[order_px] = po[lin]
            out[b, n] = flat.reshape(H, W)
    return out


# ------------------------------------------------------------- device build


def _build(cfg, k_iters, n_slots):
    P, J, TPC, NE = cfg["P"], cfg["J"], cfg["TPC"], cfg["NE"]
    RG, NPI = cfg["RG"], cfg["NPIX_INSTR"]
    EPS = cfg["EPS"]
    GMAX = max(PCOLS, 16 * n_slots[0])

    nc = bacc.Bacc("TRN2", target_bir_lowering=False, num_devices=cfg["NCORES"])
    attrs_w = nc.dram_tensor("attrs_w", [TPC, P, J, 15], f32, kind="ExternalInput")
    diff_w = nc.dram_tensor("diff_w", [TPC, P, J], f32, kind="ExternalInput")
    wgB = nc.dram_tensor("wgB", [P, TPC, 17], f32, kind="ExternalInput")
    biasB = nc.dram_tensor("biasB", [P, TPC], f32, kind="ExternalInput")
    pks = [nc.dram_tensor(f"pk{k}", [TPC, P, 2 * n_slots[k]], i16,
                          kind="ExternalInput") for k in range(k_iters)]
    p_idx = nc.dram_tensor("p_idx", [TPC * NPI, P, PCOLS // 16], i16,
                           kind="ExternalInput")
    wperm_d = nc.dram_tensor("wperm", [P, 16, P], fp16, kind="ExternalInput")
    pixout = nc.dram_tensor("pixout", [TPC * NPI, 8, 8, RG], f32,
                            kind="ExternalOutput")

    with tile.TileContext(nc) as tc:
        with (
            tc.tile_pool(name="outer", bufs=1) as opool,
            tc.tile_pool(name="dr", bufs=1, space="DRAM") as dpool,
        ):
            wg = opool.tile([P, TPC, 17], f32, tag="wg")
            nc.sync.dma_start(wg[:], wgB[:, :, :])
            bi = opool.tile([P, TPC], f32, tag="bi")
            nc.sync.dma_start(bi[:], biasB[:, :])
            epsb = opool.tile([P, 1], f32, tag="epsb")
            nc.vector.memset(epsb[:], EPS)
            hpib = opool.tile([P, 1], f32, tag="hpib")
            nc.vector.memset(hpib[:], float(np.pi / 2))
            s_tiles = [opool.tile([P, J], f32, tag=f"s{j}", name=f"s{j}")
                       for j in range(TPC)]
            wp = opool.tile([P, 16, P], fp16, tag="wp")
            nc.sync.dma_start(wp[:], wperm_d[:, :, :])

            # ---------------- features (function-batched activations,
            # two trees at a time to fit SBUF)
            for half in range(2):
              trees = [2 * half, 2 * half + 1]
              with tc.tile_pool(name="feat", bufs=1) as fp:
                at, df, lg = {}, {}, {}
                t9, s9, t1, t2 = {}, {}, {}, {}
                for j in trees:
                    a = fp.tile([P, J, 15], f32, tag=f"at{j}")
                    [nc.sync, nc.scalar, nc.gpsimd, nc.sync][j].dma_start(
                        a[:], attrs_w[j])
                    at[j] = a
                    d = fp.tile([P, J], f32, tag=f"df{j}")
                    nc.gpsimd.dma_start(d[:], diff_w[j])
                    df[j] = d
                    lg[j] = fp.tile([P, J], f32, tag=f"lg{j}", name=f"lg{j}")
                    t9[j] = fp.tile([P, J, 9], f32, tag=f"t9{j}", name=f"t9{j}")
                    s9[j] = fp.tile([P, J, 9], f32, tag=f"s9{j}", name=f"s9{j}")
                    t1[j] = fp.tile([P, J], f32, tag=f"t1{j}", name=f"t1{j}")
                    t2[j] = fp.tile([P, J], f32, tag=f"t2{j}", name=f"t2{j}")

                def wgb(j, kf):
                    return wg[:, j, kf:kf + 1]

                # abs/sign/ln tail for attrs 6..14
                for j in trees:
                    nc.scalar.activation(t9[j][:], at[j][:, :, 6:15], Act.Abs)
                for j in trees:
                    nc.scalar.activation(s9[j][:], at[j][:, :, 6:15], Act.Sign)
                for j in trees:
                    nc.scalar.activation(t9[j][:], t9[j][:], Act.Ln,
                                         bias=epsb[:, :])
                # lshape pieces
                for j in trees:
                    nc.scalar.activation(t1[j][:], at[j][:, :, 6], Act.Sqrt)
                for j in trees:
                    nc.scalar.activation(t2[j][:], at[j][:, :, 7], Act.Sqrt)
                for j in trees:
                    nc.scalar.activation(t1[j][:], t1[j][:], Act.Copy, bias=EPS)
                for j in trees:
                    nc.vector.reciprocal(t1[j][:], t1[j][:])
                for j in trees:
                    nc.vector.tensor_tensor(out=t2[j][:], in0=t2[j][:],
                                            in1=t1[j][:], op=Alu.mult)
                # tail = ln(|a|+eps)*sign
                for j in trees:
                    nc.vector.tensor_tensor(out=t9[j][:], in0=t9[j][:],
                                            in1=s9[j][:], op=Alu.mult)
                # logits accumulate: lg = f0*w0 + bias; lg += fk*wk
                for j in trees:
                    nc.vector.scalar_tensor_tensor(
                        out=lg[j][:], in0=at[j][:, :, 0], scalar=wgb(j, 0),
                        in1=bi[:, j:j + 1].to_broadcast([P, J]),
                        op0=Alu.mult, op1=Alu.add)
                    for kf in range(1, 4):
                        nc.vector.scalar_tensor_tensor(
                            out=lg[j][:], in0=at[j][:, :, kf],
                            scalar=wgb(j, kf), in1=lg[j][:],
                            op0=Alu.mult, op1=Alu.add)
                    for kf in range(9):
                        nc.vector.scalar_tensor_tensor(
                            out=lg[j][:], in0=t9[j][:, :, kf],
                            scalar=wgb(j, 5 + kf), in1=lg[j][:],
                            op0=Alu.mult, op1=Alu.add)
                    nc.vector.scalar_tensor_tensor(
                        out=lg[j][:], in0=t2[j][:], scalar=wgb(j, 14),
                        in1=lg[j][:], op0=Alu.mult, op1=Alu.add)
                # area = ln(a4)
                for j in trees:
                    nc.scalar.activation(t1[j][:], at[j][:, :, 4], Act.Ln)
                # cos = sin(a5 + pi/2); sin(a5)
                for j in trees:
                    nc.scalar.activation(t2[j][:], at[j][:, :, 5], Act.Sin,
                                         bias=hpib[:, :])
                for j in trees:
                    nc.vector.scalar_tensor_tensor(
                        out=lg[j][:], in0=t1[j][:], scalar=wgb(j, 4),
                        in1=lg[j][:], op0=Alu.mult, op1=Alu.add)
                    nc.vector.scalar_tensor_tensor(
                        out=lg[j][:], in0=t2[j][:], scalar=wgb(j, 15),
                        in1=lg[j][:], op0=Alu.mult, op1=Alu.add)
                for j in trees:
                    nc.scalar.activation(t1[j][:], at[j][:, :, 5], Act.Sin)
                for j in trees:
                    nc.vector.scalar_tensor_tensor(
                        out=lg[j][:], in0=t1[j][:], scalar=wgb(j, 16),
                        in1=lg[j][:], op0=Alu.mult, op1=Alu.add)
                # score = sigmoid(lg); w = score * diff / SCALING
                for j in trees:
                    nc.scalar.activation(lg[j][:], lg[j][:], Act.Sigmoid)
                for j in trees:
                    nc.vector.scalar_tensor_tensor(
                        out=s_tiles[j][:], in0=lg[j][:],
                        scalar=float(1.0 / cfg["SCALING"]), in1=df[j][:],
                        op0=Alu.mult, op1=Alu.mult)

            # ---------------- chase
            with (
                tc.tile_pool(name="chase", bufs=1) as cp,
                tc.tile_pool(name="ps", bufs=1, space="PSUM") as pp,
            ):
                gt = cp.tile([P, GMAX], f32, tag="gt")
                sel = cp.tile([P, J], f32, tag="sel", bufs=2)
                tmp = cp.tile([P, J], f32, tag="tmp", bufs=2)

                qnames = [nc.sync, nc.scalar, nc.gpsimd]

                def build_table(pool, s_j):
                    s16 = pool.tile([P, J], fp16, tag="s16", bufs=2, name="s16")
                    nc.scalar.activation(s16[:], s_j[:], Act.Copy)
                    tb = pool.tile([P, NE], f32, tag="tbl", bufs=2, name="tbl")
                    for r in range(16):
                        pt = pp.tile([P, J], f32, tag="pt", bufs=2, name="pt")
                        nc.tensor.matmul(pt[:], wp[:, r, :], s16[:])
                        nc.scalar.activation(tb[:, r * J:(r + 1) * J], pt[:],
                                             Act.Copy)
                    return tb

                for k in range(k_iters):
                    ns = n_slots[k]
                    cols = 16 * ns
                    for j in range(TPC):
                        tb = build_table(cp, s_tiles[j])
                        pkt = cp.tile([P, 2 * ns], i16, tag="pk", bufs=3)
                        nc.sync.dma_start(pkt[:], pks[k][j])
                        gg = cp.tile([P, GMAX], f32, tag="g", bufs=2)
                        nc.gpsimd.ap_gather(
                            out_ap=gg[:, 0:cols], in_ap=tb[:],
                            idxs_ap=pkt[:, 0:ns],
                            channels=P, num_elems=NE, d=1, num_idxs=cols)
                        nc.vector.transpose(gt[:, 0:cols], gg[:, 0:cols])
                        qs = pkt[:, ns:2 * ns].bitcast(fp16)
                        gv = gt[:, 0:cols].rearrange("p (j q) -> p j q", q=16)
                        nc.vector.scalar_tensor_tensor(
                            out=sel[:, 0:ns], in0=qs, scalar=0.0,
                            in1=gv[:, :, 0], op0=Alu.is_equal, op1=Alu.mult)
                        for q in range(1, 8):
                            nc.vector.scalar_tensor_tensor(
                                out=tmp[:, 0:ns], in0=qs, scalar=float(q),
                                in1=gv[:, :, q], op0=Alu.is_equal, op1=Alu.mult)
                            nc.vector.tensor_tensor(
                                out=sel[:, 0:ns], in0=sel[:, 0:ns],
                                in1=tmp[:, 0:ns], op=Alu.add)
                        nc.vector.tensor_tensor(
                            out=s_tiles[j][:, 0:ns], in0=s_tiles[j][:, 0:ns],
                            in1=sel[:, 0:ns], op=Alu.add)

                # ---------------- pixels
                for j in range(TPC):
                    tb = build_table(cp, s_tiles[j])
                    for c in range(NPI):
                        i = j * NPI + c
                        pit = cp.tile([P, PCOLS // 16], i16, tag="pix", bufs=3)
                        nc.sync.dma_start(pit[:], p_idx[i])
                        gg = cp.tile([P, GMAX], f32, tag="g", bufs=2)
                        nc.gpsimd.ap_gather(
                            out_ap=gg[:, 0:PCOLS], in_ap=tb[:], idxs_ap=pit[:],
                            channels=P, num_elems=NE, d=1, num_idxs=PCOLS)
                        gq = gg[:, 0:PCOLS].rearrange("(k q) c -> q k c", q=16)
                        for q in range(8):
                            eng = qnames[q % 3]
                            eng.dma_start(
                                pixout[i, q],
                                gq[q, :, q * RG:(q + 1) * RG])

    nc.compile()
    return nc


_CACHE = {}
TRACE = False
LAST_RESULT = None


def _get_nc(cfg, k_iters, n_slots):
    key = (k_iters, tuple(n_slots))
    if key not in _CACHE:
        _CACHE[key] = _build(cfg, k_iters, list(n_slots))
    return _CACHE[key]


def kernel(diff, attrs, weight, bias, parent, pix2cc):
    cfg = CFG
    diff = np.ascontiguousarray(np.asarray(diff, np.float32))
    attrs = np.ascontiguousarray(np.asarray(attrs, np.float32))
    weight = np.ascontiguousarray(np.asarray(weight, np.float32))
    bias = np.ascontiguousarray(np.asarray(bias, np.float32))
    parent = np.ascontiguousarray(np.asarray(parent, np.int32))
    pix2cc = np.ascontiguousarray(np.asarray(pix2cc, np.int32))

    in_maps, meta = _host_prep(cfg, diff, attrs, weight, bias, parent, pix2cc)
    nc = _get_nc(cfg, meta["k_iters"], meta["n_slots"])
    dev_maps = [{k: v for k, v in m.items() if k != "pk"} for m in in_maps]
    for core in range(cfg["NCORES"]):
        for k in range(meta["k_iters"]):
            dev_maps[core][f"pk{k}"] = in_maps[core]["pk"][k]
    res = run_bass_kernel_spmd(
        nc, dev_maps, core_ids=list(range(cfg["NCORES"])), trace=TRACE)
    global LAST_RESULT
    LAST_RESULT = res
    return _host_assemble(cfg, res.results, meta)



# revision 10
# speedup vs baseline: 1.8635x; 1.8635x over previous
"""Trainium2 Bass kernel for nn_DifferentialMaxtree (v2).

Strategy (8 NeuronCores, data-parallel over the 32 (b,n) trees, 4 per core):
  1. Features/logits/sigmoid/w = diff*score/SCALING on ACT+DVE (fp32).
  2. Path sums via pointer doubling with sigma=8 subtable gathers:
     - gather tables live in SBUF [128, 8192] f32 (16 replicas of the
       65536-entry image, partition t holds entries [(t%8)*8192, ...));
     - tables built by 4 parallel SBUF->DRAM writes of the s image plus
       4 parallel 1MB DRAM->SBUF replica reads (no DRAM roundtrip chains);
     - candidate co-location ON-CHIP via the DVE 32x32 StreamTranspose
       (no DRAM bounce), then an 8-way masked select with fused
       scalar_tensor_tensor ops; sentinel/pad slots use qsel=99 -> add 0.
     - components are depth-sorted so iteration k only gathers the
       active prefix (depth >= 2^k).
  3. Pixel lookups gather from the final val table; requests are sorted
     host-side by subtable-row q so results are written straight from the
     gather output to DRAM with 8 partition-strided DMAs per instruction
     (no transpose, no select). Host inverts all permutations.

All floating point math happens on device; the host only does integer
index bookkeeping (chains, depths, sorts, wrap16 layouts).
"""

import numpy as np

import concourse.bass as bass
import concourse.bacc as bacc
import concourse.mybir as mybir
import concourse.tile as tile
from concourse.bass_utils import run_bass_kernel_spmd

f32 = mybir.dt.float32
fp16 = mybir.dt.float16
i16 = mybir.dt.int16
Alu = mybir.AluOpType
Act = mybir.ActivationFunctionType

CFG = dict(
    B=4, N=8, H=512, W=512, C=65536,
    NCORES=8, TPC=4, P=128, J=512,
    NE=8192,          # table entries per partition (sigma=8)
    EPS=1e-10, SCALING=10.0,
)


# ---------------------------------------------------------------- host prep


def _host_prep(cfg, diff, attrs, weight, bias, parent, pix2cc):
    B, N, C, P, J = cfg["B"], cfg["N"], cfg["C"], cfg["P"], cfg["J"]
    NCORES, TPC, NE = cfg["NCORES"], cfg["TPC"], cfg["NE"]
    NPX = cfg["H"] * cfg["W"]

    # ---- pointer chains (sentinel C) --------------------------------------
    pz = np.concatenate([parent, np.full((B, N, 1), C, np.int32)], axis=-1)
    chains = []
    cur = pz.copy()
    k_iters = 0
    for _ in range(17):
        if (cur[..., :C] == C).all():
            break
        chains.append(cur[..., :C].copy())
        cur = np.take_along_axis(cur, cur, axis=-1)
        k_iters += 1
    if k_iters == 0:
        chains.append(cur[..., :C].copy())
        k_iters = 1

    # ---- depth via binary chain walk --------------------------------------
    depth = np.zeros((B, N, C), np.int64)
    cur_node = np.broadcast_to(np.arange(C), (B, N, C)).copy()
    for k in range(k_iters - 1, -1, -1):
        ch = np.concatenate([chains[k], np.full((B, N, 1), C, np.int32)],
                            axis=-1)
        nxt = np.take_along_axis(ch, cur_node, axis=-1)
        mask = nxt != C
        depth += mask << k
        cur_node = np.where(mask, nxt, cur_node)

    # ---- slot assignment: rank by (depth desc, pixcount desc) -------------
    # rank rho: 0 = deepest. home partition p = rho % 128, slot j = rho // 128
    # pixcount tiebreak co-locates similar pixel multiplicities in a slot
    # column, minimizing the max-over-lanes waste of the pixel stripe gather.
    pixcnt = np.zeros((B, N, C), np.int64)
    for b in range(B):
        for n in range(N):
            pixcnt[b, n] = np.bincount(pix2cc[b, n].reshape(-1), minlength=C)
    order = np.empty((B, N, C), np.int64)
    rho = np.empty_like(order)
    ar = np.arange(C)
    for b in range(B):
        for n in range(N):
            order[b, n] = np.lexsort((-pixcnt[b, n], -depth[b, n]))
            rho[b, n, order[b, n]] = ar
    home_p = rho % P                                         # [B,N,C]
    home_j = rho // P
    tpos = home_p * J + home_j                               # table position

    # active slot counts per iteration (global max over trees/partitions)
    n_slots = []
    for k in range(k_iters):
        n_act = (depth >= (1 << k)).sum(axis=-1)             # [B,N]
        mx = int(np.ceil(n_act.max() / P))
        mx = max(2, mx + (mx & 1))                           # even, >=2
        mx = min(mx, J)
        n_slots.append(mx)

    # ---- request/tile layout helpers --------------------------------------
    # slot (p, j) -> request (core, i): core = 2*(p//32) + j%2,
    #                                   i = 32*(j//2) + p%32
    # request (core, i) -> idx tile cell (16*core + i%16, i//16)
    pgrid = np.arange(P)[:, None]
    tpos_pad = np.concatenate(
        [tpos, np.zeros((B, N, 1), tpos.dtype)], axis=-1)    # tpos[C] unused pad

    in_maps = [dict() for _ in range(NCORES)]

    # ---- per-tree chase inputs --------------------------------------------
    # packed per (tree, iter): i16 [P, 2*ns]: [:, :ns] idx, [:, ns:] qsel fp16
    for core in range(NCORES):
        packs = []
        for k in range(k_iters):
            ns = n_slots[k]
            pk = np.zeros((TPC, P, 2 * ns), np.int16)
            packs.append(pk)
        in_maps[core]["pk"] = packs

    # comp_at[p, j] = component id at slot (p, j), per tree
    for t in range(B * N):
        b, n = t // N, t % N
        core, j_tree = t // TPC, t % TPC
        comp_at = np.empty((P, J), np.int64)
        comp_at[home_p[b, n], home_j[b, n]] = np.arange(C)
        for k in range(k_iters):
            ns = n_slots[k]
            jgrid = np.arange(ns)[None, :]
            comps = comp_at[:, :ns]                          # [P, ns]
            targ = chains[k][b, n][comps]                    # [P, ns] in [0,C]
            act = targ != C
            tp = tpos_pad[b, n][np.where(act, targ, C)]
            qv = np.where(act, tp // NE, 99).astype(np.float16)
            iv = np.where(act, tp % NE, 0).astype(np.int16)
            # scatter into wrapped idx tile + qsel plane
            kcore = 2 * (pgrid // 32) + (jgrid % 2)          # [P, ns]
            i_req = 32 * (jgrid // 2) + (pgrid % 32)
            tile_p = 16 * kcore + (i_req % 16)
            tile_o = i_req // 16
            pk = in_maps[core]["pk"][k][j_tree]
            pk[tile_p, tile_o] = iv
            pk[:, ns:] = qv.view(np.int16)

    # ---- features inputs ---------------------------------------------------
    for core in range(NCORES):
        attrs_w = np.empty((TPC, P, J, 15), np.float32)
        diff_w = np.empty((TPC, P, J), np.float32)
        wgB = np.empty((P, TPC, 17), np.float32)
        biasB = np.empty((P, TPC), np.float32)
        for j_tree in range(TPC):
            t = core * TPC + j_tree
            b, n = t // N, t % N
            comp_at = np.empty((P, J), np.int64)
            comp_at[home_p[b, n], home_j[b, n]] = np.arange(C)
            attrs_w[j_tree] = attrs[b, n][comp_at]
            diff_w[j_tree] = diff[b, n][comp_at]
            wgB[:, j_tree, :] = weight[n, :, 0][None, :]
            biasB[:, j_tree] = bias[n, 0]
        in_maps[core]["attrs_w"] = attrs_w
        in_maps[core]["diff_w"] = diff_w
        in_maps[core]["wgB"] = wgB
        in_maps[core]["biasB"] = biasB

    # permutation weights for table-build matmuls:
    # table[t, r*J + e0] = s[(t%8)*16 + r, e0]  ->  W_r[pi, po] = [pi == (po%8)*16+r]
    wperm = np.zeros((P, 16, P), np.float16)
    po = np.arange(P)
    for r in range(16):
        wperm[(po % 8) * 16 + r, r, po] = 1.0
    for core in range(NCORES):
        in_maps[core]["wperm"] = wperm

    # ---- pixel inputs: stripe gather straight from the s tiles -------------
    # Bucket pixel requests by target slot column: gather column i of Q7
    # core k fetches s[16k+m, jcol(i)] for all 16 lanes m; the pixel copy
    # beta of the component at slot (p, jcol) is read from lane p%16 at
    # column pbase[p//16, jcol] + beta. Delivery order is free (host
    # inverse-permutes during assembly), so no transpose/select is needed.
    pxdata = {}
    pxcols_max = 0
    for b in range(B):
        for n in range(N):
            cnt = pixcnt[b, n]
            Pp, Jj = home_p[b, n], home_j[b, n]
            colneed = np.zeros((8, J), np.int64)
            np.maximum.at(colneed, (Pp // 16, Jj), cnt)
            pbase = np.zeros((8, J), np.int64)
            pbase[:, 1:] = np.cumsum(colneed, axis=1)[:, :-1]
            pxdata[(b, n)] = (colneed, pbase)
            pxcols_max = max(pxcols_max, int(colneed.sum(axis=1).max()))
    PXCOLS = int(np.ceil(pxcols_max / 16) * 16)

    asm = []   # per core: list of lin arrays (pixel-order index into pixout)
    for core in range(NCORES):
        p_idx = np.zeros((TPC, P, PXCOLS // 16), np.int16)
        lin_all = []
        for j_tree in range(TPC):
            t = core * TPC + j_tree
            b, n = t // N, t % N
            colneed, pbase = pxdata[(b, n)]
            pix = pix2cc[b, n].reshape(-1)                   # [NPX]
            Pp, Jj = home_p[b, n], home_j[b, n]
            for k in range(8):
                jvals = np.repeat(np.arange(J, dtype=np.int16), colneed[k])
                ii = np.arange(len(jvals))
                p_idx[j_tree, 16 * k + ii % 16, ii // 16] = jvals
            order_px = np.argsort(pix, kind="stable")
            pix_s = pix[order_px]
            beta = np.arange(NPX) - np.searchsorted(pix_s, pix_s)
            pcol = pbase[Pp[pix_s] // 16, Jj[pix_s]] + beta
            lin_sorted = Pp[pix_s] * PXCOLS + pcol
            lin = np.empty(NPX, np.int64)
            lin[order_px] = lin_sorted
            lin_all.append(lin)
        in_maps[core]["p_idx"] = p_idx
        asm.append(lin_all)

    meta = dict(k_iters=k_iters, n_slots=tuple(n_slots), asm=asm,
                PXCOLS=PXCOLS)
    return in_maps, meta


def _host_assemble(cfg, results, meta):
    B, N, H, W = cfg["B"], cfg["N"], cfg["H"], cfg["W"]
    NCORES, TPC = cfg["NCORES"], cfg["TPC"]
    out = np.empty((B, N, H, W), np.float32)
    for core in range(NCORES):
        po = results[core]["pixout"]                         # [TPC, P, PXCOLS]
        for j_tree in range(TPC):
            t = core * TPC + j_tree
            b, n = t // N, t % N
            lin = meta["asm"][core][j_tree]
            out[b, n] = po[j_tree].reshape(-1)[lin].reshape(H, W)
    return out


# ------------------------------------------------------------- device build


def _build(cfg, k_iters, n_slots, pxcols):
    P, J, TPC, NE = cfg["P"], cfg["J"], cfg["TPC"], cfg["NE"]
    EPS = cfg["EPS"]
    GMAX = 16 * n_slots[0]

    nc = bacc.Bacc("TRN2", target_bir_lowering=False, num_devices=cfg["NCORES"])
    attrs_w = nc.dram_tensor("attrs_w", [TPC, P, J, 15], f32, kind="ExternalInput")
    diff_w = nc.dram_tensor("diff_w", [TPC, P, J], f32, kind="ExternalInput")
    wgB = nc.dram_tensor("wgB", [P, TPC, 17], f32, kind="ExternalInput")
    biasB = nc.dram_tensor("biasB", [P, TPC], f32, kind="ExternalInput")
    pks = [nc.dram_tensor(f"pk{k}", [TPC, P, 2 * n_slots[k]], i16,
                          kind="ExternalInput") for k in range(k_iters)]
    p_idx = nc.dram_tensor("p_idx", [TPC, P, pxcols // 16], i16,
                           kind="ExternalInput")
    wperm_d = nc.dram_tensor("wperm", [P, 16, P], fp16, kind="ExternalInput")
    pixout = nc.dram_tensor("pixout", [TPC, P, pxcols], f32,
                            kind="ExternalOutput")

    with tile.TileContext(nc) as tc:
        with (
            tc.tile_pool(name="outer", bufs=1) as opool,
            tc.tile_pool(name="dr", bufs=1, space="DRAM") as dpool,
        ):
            wg = opool.tile([P, TPC, 17], f32, tag="wg")
            nc.sync.dma_start(wg[:], wgB[:, :, :])
            bi = opool.tile([P, TPC], f32, tag="bi")
            nc.sync.dma_start(bi[:], biasB[:, :])
            epsb = opool.tile([P, 1], f32, tag="epsb")
            nc.vector.memset(epsb[:], EPS)
            hpib = opool.tile([P, 1], f32, tag="hpib")
            nc.vector.memset(hpib[:], float(np.pi / 2))
            s_tiles = [opool.tile([P, J], f32, tag=f"s{j}", name=f"s{j}")
                       for j in range(TPC)]
            wp = opool.tile([P, 16, P], fp16, tag="wp")
            nc.sync.dma_start(wp[:], wperm_d[:, :, :])

            # ---------------- features (function-batched activations,
            # two trees at a time to fit SBUF)
            for half in range(2):
              trees = [2 * half, 2 * half + 1]
              with tc.tile_pool(name="feat", bufs=1) as fp:
                at, df, lg = {}, {}, {}
                t9, s9, t1, t2 = {}, {}, {}, {}
                for j in trees:
                    a = fp.tile([P, J, 15], f32, tag=f"at{j}")
                    [nc.sync, nc.scalar, nc.gpsimd, nc.sync][j].dma_start(
                        a[:], attrs_w[j])
                    at[j] = a
                    d = fp.tile([P, J], f32, tag=f"df{j}")
                    nc.gpsimd.dma_start(d[:], diff_w[j])
                    df[j] = d
                    lg[j] = fp.tile([P, J], f32, tag=f"lg{j}", name=f"lg{j}")
                    t9[j] = fp.tile([P, J, 9], f32, tag=f"t9{j}", name=f"t9{j}")
                    s9[j] = fp.tile([P, J, 9], f32, tag=f"s9{j}", name=f"s9{j}")
                    t1[j] = fp.tile([P, J], f32, tag=f"t1{j}", name=f"t1{j}")
                    t2[j] = fp.tile([P, J], f32, tag=f"t2{j}", name=f"t2{j}")

                def wgb(j, kf):
                    return wg[:, j, kf:kf + 1]

                # abs/sign/ln tail for attrs 6..14
                for j in trees:
                    nc.scalar.activation(t9[j][:], at[j][:, :, 6:15], Act.Abs)
                for j in trees:
                    nc.scalar.activation(s9[j][:], at[j][:, :, 6:15], Act.Sign)
                for j in trees:
                    nc.scalar.activation(t9[j][:], t9[j][:], Act.Ln,
                                         bias=epsb[:, :])
                # lshape pieces
                for j in trees:
                    nc.scalar.activation(t1[j][:], at[j][:, :, 6], Act.Sqrt)
                for j in trees:
                    nc.scalar.activation(t2[j][:], at[j][:, :, 7], Act.Sqrt)
                for j in trees:
                    nc.scalar.activation(t1[j][:], t1[j][:], Act.Copy, bias=EPS)
                for j in trees:
                    nc.vector.reciprocal(t1[j][:], t1[j][:])
                for j in trees:
                    nc.vector.tensor_tensor(out=t2[j][:], in0=t2[j][:],
                                            in1=t1[j][:], op=Alu.mult)
                # tail = ln(|a|+eps)*sign
                for j in trees:
                    nc.vector.tensor_tensor(out=t9[j][:], in0=t9[j][:],
                                            in1=s9[j][:], op=Alu.mult)
                # logits accumulate: lg = f0*w0 + bias; lg += fk*wk
                for j in trees:
                    nc.vector.scalar_tensor_tensor(
                        out=lg[j][:], in0=at[j][:, :, 0], scalar=wgb(j, 0),
                        in1=bi[:, j:j + 1].to_broadcast([P, J]),
                        op0=Alu.mult, op1=Alu.add)
                    for kf in range(1, 4):
                        nc.vector.scalar_tensor_tensor(
                            out=lg[j][:], in0=at[j][:, :, kf],
                            scalar=wgb(j, kf), in1=lg[j][:],
                            op0=Alu.mult, op1=Alu.add)
                    for kf in range(9):
                        nc.vector.scalar_tensor_tensor(
                            out=lg[j][:], in0=t9[j][:, :, kf],
                            scalar=wgb(j, 5 + kf), in1=lg[j][:],
                            op0=Alu.mult, op1=Alu.add)
                    nc.vector.scalar_tensor_tensor(
                        out=lg[j][:], in0=t2[j][:], scalar=wgb(j, 14),
                        in1=lg[j][:], op0=Alu.mult, op1=Alu.add)
                # area = ln(a4)
                for j in trees:
                    nc.scalar.activation(t1[j][:], at[j][:, :, 4], Act.Ln)
                # cos = sin(a5 + pi/2); sin(a5)
                for j in trees:
                    nc.scalar.activation(t2[j][:], at[j][:, :, 5], Act.Sin,
                                         bias=hpib[:, :])
                for j in trees:
                    nc.vector.scalar_tensor_tensor(
                        out=lg[j][:], in0=t1[j][:], scalar=wgb(j, 4),
                        in1=lg[j][:], op0=Alu.mult, op1=Alu.add)
                    nc.vector.scalar_tensor_tensor(
                        out=lg[j][:], in0=t2[j][:], scalar=wgb(j, 15),
                        in1=lg[j][:], op0=Alu.mult, op1=Alu.add)
                for j in trees:
                    nc.scalar.activation(t1[j][:], at[j][:, :, 5], Act.Sin)
                for j in trees:
                    nc.vector.scalar_tensor_tensor(
                        out=lg[j][:], in0=t1[j][:], scalar=wgb(j, 16),
                        in1=lg[j][:], op0=Alu.mult, op1=Alu.add)
                # score = sigmoid(lg); w = score * diff / SCALING
                for j in trees:
                    nc.scalar.activation(lg[j][:], lg[j][:], Act.Sigmoid)
                for j in trees:
                    nc.vector.scalar_tensor_tensor(
                        out=s_tiles[j][:], in0=lg[j][:],
                        scalar=float(1.0 / cfg["SCALING"]), in1=df[j][:],
                        op0=Alu.mult, op1=Alu.mult)

            # ---------------- chase
            with (
                tc.tile_pool(name="chase", bufs=1) as cp,
                tc.tile_pool(name="ps", bufs=1, space="PSUM") as pp,
            ):
                gt = cp.tile([P, GMAX], f32, tag="gt")
                sel = cp.tile([P, J], f32, tag="sel", bufs=2)
                tmp = cp.tile([P, J], f32, tag="tmp", bufs=2)

                qnames = [nc.sync, nc.scalar, nc.gpsimd]

                def build_table(pool, s_j):
                    s16 = pool.tile([P, J], fp16, tag="s16", bufs=2, name="s16")
                    nc.scalar.activation(s16[:], s_j[:], Act.Copy)
                    tb = pool.tile([P, NE], f32, tag="tbl", bufs=2, name="tbl")
                    for r in range(16):
                        pt = pp.tile([P, J], f32, tag="pt", bufs=2, name="pt")
                        nc.tensor.matmul(pt[:], wp[:, r, :], s16[:])
                        nc.scalar.activation(tb[:, r * J:(r + 1) * J], pt[:],
                                             Act.Copy)
                    return tb

                for k in range(k_iters):
                    ns = n_slots[k]
                    cols = 16 * ns
                    for j in range(TPC):
                        tb = build_table(cp, s_tiles[j])
                        pkt = cp.tile([P, 2 * ns], i16, tag="pk", bufs=3)
                        nc.sync.dma_start(pkt[:], pks[k][j])
                        gg = cp.tile([P, GMAX], f32, tag="g", bufs=2)
                        nc.gpsimd.ap_gather(
                            out_ap=gg[:, 0:cols], in_ap=tb[:],
                            idxs_ap=pkt[:, 0:ns],
                            channels=P, num_elems=NE, d=1, num_idxs=cols)
                        nc.vector.transpose(gt[:, 0:cols], gg[:, 0:cols])
                        qs = pkt[:, ns:2 * ns].bitcast(fp16)
                        gv = gt[:, 0:cols].rearrange("p (j q) -> p j q", q=16)
                        nc.vector.scalar_tensor_tensor(
                            out=sel[:, 0:ns], in0=qs, scalar=0.0,
                            in1=gv[:, :, 0], op0=Alu.is_equal, op1=Alu.mult)
                        for q in range(1, 8):
                            nc.vector.scalar_tensor_tensor(
                                out=tmp[:, 0:ns], in0=qs, scalar=float(q),
                                in1=gv[:, :, q], op0=Alu.is_equal, op1=Alu.mult)
                            nc.vector.tensor_tensor(
                                out=sel[:, 0:ns], in0=sel[:, 0:ns],
                                in1=tmp[:, 0:ns], op=Alu.add)
                        nc.vector.tensor_tensor(
                            out=s_tiles[j][:, 0:ns], in0=s_tiles[j][:, 0:ns],
                            in1=sel[:, 0:ns], op=Alu.add)

                # ---------------- pixels: stripe gather straight from s
                for j in range(TPC):
                    pit = cp.tile([P, pxcols // 16], i16, tag="pix", bufs=2)
                    nc.sync.dma_start(pit[:], p_idx[j])
                    gpx = cp.tile([P, pxcols], f32, tag="gpx", bufs=2)
                    nc.gpsimd.ap_gather(
                        out_ap=gpx[:], in_ap=s_tiles[j][:], idxs_ap=pit[:],
                        channels=P, num_elems=J, d=1, num_idxs=pxcols)
                    [nc.sync, nc.scalar][j % 2].dma_start(pixout[j], gpx[:])

    nc.compile()
    return nc


_CACHE = {}
TRACE = False
LAST_RESULT = None


def _get_nc(cfg, k_iters, n_slots, pxcols):
    key = (k_iters, tuple(n_slots), pxcols)
    if key not in _CACHE:
        _CACHE[key] = _build(cfg, k_iters, list(n_slots), pxcols)
    return _CACHE[key]


def kernel(diff, attrs, weight, bias, parent, pix2cc):
    cfg = CFG
    diff = np.ascontiguousarray(np.asarray(diff, np.float32))
    attrs = np.ascontiguousarray(np.asarray(attrs, np.float32))
    weight = np.ascontiguousarray(np.asarray(weight, np.float32))
    bias = np.ascontiguousarray(np.asarray(bias, np.float32))
    parent = np.ascontiguousarray(np.asarray(parent, np.int32))
    pix2cc = np.ascontiguousarray(np.asarray(pix2cc, np.int32))

    in_maps, meta = _host_prep(cfg, diff, attrs, weight, bias, parent, pix2cc)
    nc = _get_nc(cfg, meta["k_iters"], meta["n_slots"], meta["PXCOLS"])
    dev_maps = [{k: v for k, v in m.items() if k != "pk"} for m in in_maps]
    for core in range(cfg["NCORES"]):
        for k in range(meta["k_iters"]):
            dev_maps[core][f"pk{k}"] = in_maps[core]["pk"][k]
    res = run_bass_kernel_spmd(
        nc, dev_maps, core_ids=list(range(cfg["NCORES"])), trace=TRACE)
    global LAST_RESULT
    LAST_RESULT = res
    return _host_assemble(cfg, res.results, meta)

